# revision 115
# baseline (speedup 1.0000x reference)
"""Trainium2 Bass kernel for the Involution module (B=8, C=256, H=W=56, K=7).

Strategy (8 NeuronCores, data-parallel over batch):
  Each core processes one batch element.
  - conv1x1+BN+ReLU twice on the PE (bf16, BN folded into weights in numpy).
  - Involution: partitions = (group g:16, kj:7) = 112 lanes.
    x is pre-replicated 7x (kj-shifted copies) host-side -> streamed into
    SBUF in 4 progressive row-chunks so band 0's products start early.
    For each tap row ki: DVE computes products wgt[g,ki*7+kj,hw] * x[(g,i),hw]
    (bf16 tensor_tensor, wgt broadcast over i via stride-0 AP).
    PE reduces over kj with a 0/1 selection matmul, accumulating the 7 ki
    iterations in PSUM. ACT copies PSUM->SBUF (bf16), DMA writes compact
    bf16 output.
"""

import numpy as np
import ml_dtypes

B, C, H, W = 8, 256, 56, 56
K = 7
GC = 16
G = 16
RED = 64
K2 = 49
EPS = 1e-5
HW = H * W            # 3136
PAD = 3
HP = H + 2 * PAD      # 62
WP = W + 2 * PAD      # 62
NB = 14               # bands
BW = HW // NB         # 224 columns per band (4 output rows)
BR = 4                # rows per band
NKJ = 7
NP = G * NKJ          # 112 partitions

# i-pair p4 -> PSUM strip (j, s). Chosen so each product-quarter's two
# selection matmuls land in different PE quadrants (tile_position pipelining);
# the host undoes the resulting channel permutation.
def _strip(p4):
    quarter, qq = p4 // 2, p4 % 2
    return 2 * qq + quarter % 2, quarter // 2


# x_rep row-range loads (r0, r1) into one resident SBUF tile laid out
# [GC, rows, W] per partition (i-major keeps the DVE product APs contiguous).
# Band b, tap-row ki reads input rows 4b+ki .. 4b+ki+3; finer granularity up
# front lets band 0 start early, ranges are overlap-free.
# rows 0..6 also live in a dedicated contiguous band-0 tile (one DMA
# descriptor per partition instead of 16, so the first product starts sooner);
# xfull holds rows 4..61
XB0R = 7
XF0 = 4
CHUNKS = [(4, 10), (10, 14), (14, 18), (18, 26), (26, 34), (34, 48), (48, 62)]

bf16 = ml_dtypes.bfloat16

_CACHE = {}


def _build_nc():
    import concourse.bacc as bacc
    import concourse.tile as tile
    from concourse import mybir

    f32 = mybir.dt.float32
    b16 = mybir.dt.bfloat16

    nc = bacc.Bacc("TRN2", target_bir_lowering=False, debug=False, num_devices=8)

    # per-chunk x_rep tensors, laid out exactly like their SBUF destination so
    # each partition is one contiguous DMA run
    x_rep = [
        nc.dram_tensor(
            f"x_rep{c}", [NP, (r1 - r0) * GC * W], b16, kind="ExternalInput"
        ).ap()
        for c, (r0, r1) in enumerate(CHUNKS)
    ]
    x_b0 = nc.dram_tensor("x_b0", [NP, GC, XB0R, W], b16, kind="ExternalInput").ap()
    x_conv = nc.dram_tensor("x_conv", [128, 2, HW], b16, kind="ExternalInput").ap()
    # involution weights for bands 0/1 precomputed host-side so band 0's
    # products start without waiting for the on-chip conv chain
    wgh = [
        nc.dram_tensor(f"wgh{b}", [NP, K, BW], b16, kind="ExternalInput").ap()
        for b in range(4)
    ]
    w1t = nc.dram_tensor("w1t", [128, 2, RED], b16, kind="ExternalInput").ap()
    w2t = nc.dram_tensor("w2t", [RED, K, NP], b16, kind="ExternalInput").ap()
    sel = nc.dram_tensor("sel", [NP, G], b16, kind="ExternalInput").ap()
    b1 = nc.dram_tensor("b1", [RED, 1], f32, kind="ExternalInput").ap()
    b2 = nc.dram_tensor("b2", [NP, K], f32, kind="ExternalInput").ap()
    # out compact bf16: [j, g, i4, hw]; channel = 16g + 4j + i4
    out = nc.dram_tensor("out", [4, G, 4, HW], b16, kind="ExternalOutput").ap()

    with tile.TileContext(nc) as tc:
        _body(tc, nc, mybir, x_rep, x_b0, x_conv, wgh, w1t, w2t, sel, b1, b2, out)

    nc.compile()
    return nc


def _body(tc, nc, mybir, x_rep, x_b0, x_conv, wgh, w1t, w2t, sel, b1, b2, out):
    f32 = mybir.dt.float32
    b16 = mybir.dt.bfloat16
    Relu = mybir.ActivationFunctionType.Relu
    mult = mybir.AluOpType.mult

    import contextlib
    ctx = contextlib.ExitStack()
    const = ctx.enter_context(tc.tile_pool(name="const", bufs=1))
    xrp = ctx.enter_context(tc.tile_pool(name="xrp", bufs=1))
    h1p = ctx.enter_context(tc.tile_pool(name="h1p", bufs=3))
    wgp = ctx.enter_context(tc.tile_pool(name="wgp", bufs=3))
    prp = ctx.enter_context(tc.tile_pool(name="prp", bufs=2))
    osp = ctx.enter_context(tc.tile_pool(name="osp", bufs=3))
    ph1 = ctx.enter_context(tc.tile_pool(name="ph1", bufs=2, space="PSUM"))
    ph2 = ctx.enter_context(tc.tile_pool(name="ph2", bufs=2, space="PSUM"))
    pho = ctx.enter_context(tc.tile_pool(name="pho", bufs=2, space="PSUM"))

    # ---- resident constants + all conv input, queued first ----
    # everything the band loop needs from HBM besides x_rep chunks is loaded
    # upfront: any DMA enqueued later would sit behind megabytes of chunk
    # traffic in the queues (strict FIFO) and stall its consumer.
    w1s = const.tile([128, 2, RED], b16, tag="w1s")
    nc.sync.dma_start(out=w1s[:], in_=w1t)
    b1s = const.tile([RED, 1], f32, tag="b1s")
    nc.sync.dma_start(out=b1s[:], in_=b1)

    # host-precomputed involution weights for bands 0/1
    wgh0 = const.tile([NP, K, BW], b16, tag="wgh0")
    nc.sync.dma_start(out=wgh0[:], in_=wgh[0])

    # band-0 rows as one contiguous per-partition DMA (fast first product)
    xb0 = xrp.tile([NP, GC, XB0R, W], b16, tag="xb0")
    nc.sync.dma_start(out=xb0[:], in_=x_b0)

    # resident x tile for rows XF0..61, [GC, rows, W] per partition; row-range
    # DMAs fill it progressively and subtile deps gate each band on its rows
    xfull = xrp.tile([NP, GC, HP - XF0, W], b16, tag="xfull")

    def load_chunk(cidx):
        r0, r1 = CHUNKS[cidx]
        nc.sync.dma_start(
            out=xfull[:, :, r0 - XF0:r1 - XF0, :],
            in_=x_rep[cidx].rearrange("p (i r n) -> p i r n", i=GC, r=r1 - r0),
        )

    load_chunk(0)

    sels = const.tile([NP, G], b16, tag="sels")
    nc.sync.dma_start(out=sels[:], in_=sel)

    load_chunk(1)

    wgh1 = const.tile([NP, K, BW], b16, tag="wgh1")
    nc.sync.dma_start(out=wgh1[:], in_=wgh[1])

    load_chunk(2)

    wgh2 = const.tile([NP, K, BW], b16, tag="wgh2")
    nc.sync.dma_start(out=wgh2[:], in_=wgh[2])

    load_chunk(3)

    wgh3 = const.tile([NP, K, BW], b16, tag="wgh3")
    nc.sync.dma_start(out=wgh3[:], in_=wgh[3])

    w2s = const.tile([RED, K, NP], b16, tag="w2s")
    nc.sync.dma_start(out=w2s[:], in_=w2t)
    b2s = const.tile([NP, K], f32, tag="b2s")
    nc.sync.dma_start(out=b2s[:], in_=b2)

    xcs = const.tile([128, 2, HW], b16, tag="xcs")
    nc.sync.dma_start(out=xcs[:, :, 2 * BW:4 * BW], in_=x_conv[:, :, 2 * BW:4 * BW])

    load_chunk(4)
    nc.sync.dma_start(out=xcs[:, :, 4 * BW:HW], in_=x_conv[:, :, 4 * BW:HW])
    for cidx in range(5, len(CHUNKS)):
        load_chunk(cidx)

    # conv stage for band b: conv1 + conv2 + ReLUs producing wgb[b].
    # Emitted one band ahead of the involution so the PE/ACT instructions for
    # band b+1 sit in front of band b's selection matmuls in the engine queues.
    wgbs = {}
    h1bs = {}

    def emit_conv1(b):
        n0 = b * BW
        p1 = ph1.tile([RED, BW], f32, tag="p1")
        nc.tensor.matmul(p1[:], w1s[:, 0, :], xcs[:, 0, n0:n0 + BW], start=True, stop=False)
        nc.tensor.matmul(p1[:], w1s[:, 1, :], xcs[:, 1, n0:n0 + BW], start=False, stop=True)
        h1b = h1p.tile([RED, BW], b16, tag="h1b")
        nc.scalar.activation(h1b[:], p1[:], Relu, bias=b1s[:], scale=1.0)
        h1bs[b] = h1b
        wgb = wgp.tile([NP, K, BW], b16, tag="wgb")
        wgbs[b] = wgb

    def emit_conv2(b, kis):
        h1b, wgb = h1bs[b], wgbs[b]
        for ki in kis:
            p2 = ph2.tile([NP, BW], f32, tag="p2")
            nc.tensor.matmul(p2[:], w2s[:, ki, :], h1b[:], start=True, stop=True)
            nc.scalar.activation(
                wgb[:, ki, :], p2[:], Relu, bias=b2s[:, ki:ki + 1], scale=1.0
            )
        if kis[-1] == K - 1:
            del h1bs[b]

    def emit_conv(b):
        emit_conv1(b)
        emit_conv2(b, range(K))

    wgbs[0] = wgh0
    wgbs[1] = wgh1
    wgbs[2] = wgh2
    wgbs[3] = wgh3

    for b in range(NB):
        n0 = b * BW
        wgb = wgbs.pop(b)

        # involution: products + kj/ki reduction.
        # Band 0 runs one DVE op per tap row so it can start as soon as the
        # first rows land; later bands merge all 7 tap rows into one DVE op
        # per group-channel half (sliding-window AP), saving per-op overhead.
        po = pho.tile([128, 2, 512], f32, tag="po")  # s-slot padded to one PSUM bank
        if b == 0 or b == NB - 1:
            # tap rows that read zero padding (band 0 top / band 13 bottom)
            # only compute/accumulate their non-pad column blocks; a full-span
            # tap goes first to own the PSUM start flag. zr(ki) = (z0, z1)
            # zero row-blocks at the top/bottom of the 4-row window.
            if b == 0:
                kis = (3, 0, 1, 2, 4, 5, 6)
                zr = lambda ki: (max(0, 3 - ki), 0)
            else:
                kis = tuple(range(K))
                zr = lambda ki: (0, max(0, ki - 3))
            for idx, ki in enumerate(kis):
                r = BR * b + ki
                z0, z1 = zr(ki)
                c0, c1 = 56 * z0, BW - 56 * z1
                rl, rh = r + z0, r + BR - z1
                pr = prp.tile([NP, GC, BW], b16, tag="pr", bufs=2)
                if rh <= XB0R:
                    in0 = xb0[:, :, rl:rh, :].rearrange("p i r n -> p i (r n)")
                else:
                    in0 = xfull[:, :, rl - XF0:rh - XF0, :].rearrange("p i r n -> p i (r n)")
                in1 = wgb[:, ki, c0:c1].unsqueeze(1).broadcast_to([NP, GC, c1 - c0])
                nc.vector.tensor_tensor(out=pr[:, :, c0:c1], in0=in0, in1=in1, op=mult)
                # at the final stop row of the last band, finish strips in
                # (j=0,2,1,3) order so the DVE casts can start earliest
                p4s = (0, 4, 1, 5, 2, 6, 3, 7) if (b == NB - 1 and idx == K - 1) else range(8)
                for p4 in p4s:
                    j, s = _strip(p4)
                    nc.tensor.matmul(
                        po[32 * j:32 * j + G, s, 0:2 * BW]
                        .rearrange("p (r n) -> p r n", n=BW)[:, :, c0:c1],
                        sels[:],
                        pr[:, 2 * p4:2 * p4 + 2, c0:c1],
                        start=(idx == 0),
                        stop=(idx == K - 1),
                        tile_position=(0, 32 * j),
                    )
        else:
            r0 = BR * b
            base = xfull[:]
            pstride, pnum = base.ap[0]
            for quarter in range(4):
                ih = 4 * quarter
                pr = prp.tile([NP, K, 4, BW], b16, tag="prm", bufs=3)
                in0 = type(base)(
                    base.tensor,
                    base.offset + (ih * (HP - XF0) + r0 - XF0) * W,
                    [[pstride, pnum], [W, K], [(HP - XF0) * W, 4], [1, BW]],
                )
                in1 = wgb[:].unsqueeze(2).broadcast_to([NP, K, 4, BW])
                nc.vector.tensor_tensor(out=pr[:], in0=in0, in1=in1, op=mult)
                if quarter == 0 and b >= 2 and b + 2 < NB:
                    emit_conv(b + 2)
                for ki in range(K):
                    for q in range(2):
                        j, s = _strip(2 * quarter + q)
                        nc.tensor.matmul(
                            po[32 * j:32 * j + G, s, 0:2 * BW],
                            sels[:],
                            pr[:, ki, 2 * q:2 * q + 2, :],
                            start=(ki == 0),
                            stop=(ki == K - 1),
                            tile_position=(0, 32 * j),
                        )

        # PSUM -> SBUF (bf16) -> HBM compact   (ob rows p=32j+g, free (i4=2s+r, hw))
        # last band: split copies across ACT and the now-idle DVE to shrink the tail
        ob = osp.tile([128, 4, BW], b16, tag="ob")
        # strip completion order is (0, 2, 1, 3); on the last band ACT takes the
        # first-firing strips (its sequencer wakeup is ~2.6us post-signal) and
        # the DVE casts take the later pair in parallel
        for j in (0, 2, 1, 3):
            o_ap = ob[32 * j:32 * j + G, :, :].rearrange("p (s r) n -> p s r n", s=2)
            i_ap = po[32 * j:32 * j + G, :, 0:2 * BW].rearrange("p s (r n) -> p s r n", r=2)
            if b == NB - 1 and j in (1, 3):
                nc.vector.tensor_copy(out=o_ap, in_=i_ap)
            else:
                nc.scalar.copy(out=o_ap, in_=i_ap)
        for j in (0, 1, 2, 3):
            nc.sync.dma_start(
                out=out[j, :, :, n0:n0 + BW], in_=ob[32 * j:32 * j + G, :, :]
            )

    ctx.close()


def _prep_weights(w1, b1, g1, be1, m1, v1, w2, b2, g2, be2, m2, v2):
    s1 = (g1 / np.sqrt(v1 + EPS)).astype(np.float64)
    W1p = w1.astype(np.float64) * s1[:, None]
    b1p = be1 + (b1 - m1) * (g1 / np.sqrt(v1 + EPS))
    s2 = (g2 / np.sqrt(v2 + EPS)).astype(np.float64)
    W2p = w2.astype(np.float64) * s2[:, None]
    b2p = be2 + (b2 - m2) * (g2 / np.sqrt(v2 + EPS))

    w1t = np.ascontiguousarray(
        W1p.astype(np.float32).T.reshape(2, 128, RED).transpose(1, 0, 2)
    ).astype(bf16)
    # w2t[r, ki, 7g+kj] = W2p[g*49 + ki*7 + kj, r]
    w2t = np.ascontiguousarray(
        W2p.astype(np.float32).reshape(G, K, K, RED).transpose(3, 1, 0, 2).reshape(RED, K, NP)
    ).astype(bf16)
    b2t = np.ascontiguousarray(
        b2p.astype(np.float32).reshape(G, K, K).transpose(0, 2, 1).reshape(NP, K)
    )
    selm = np.repeat(np.eye(G, dtype=np.float32), NKJ, axis=0).astype(bf16)
    dense = (
        W1p.astype(np.float32), b1p.astype(np.float32),
        W2p.astype(np.float32), b2p.astype(np.float32),
    )
    return (
        w1t,
        b1p.astype(np.float32).reshape(RED, 1),
        w2t,
        b2t,
        selm,
        dense,
    )


NWGH = 4


def _host_wgh(xc, dense):
    """Involution weights for bands 0..3 (cols 0..4*BW), laid out [7g+kj, ki, c]."""
    W1p, b1p, W2p, b2p = dense
    xcols = xc.reshape(C, HW)[:, :NWGH * BW]
    h1 = np.maximum(W1p @ xcols + b1p[:, None], 0.0)
    wg = np.maximum(W2p @ h1 + b2p[:, None], 0.0)          # [784, 4*BW]
    wg = wg.reshape(G, K, K, NWGH, BW).transpose(0, 2, 3, 1, 4)  # [g, kj, b, ki, c]
    return np.ascontiguousarray(wg.reshape(NP, NWGH, K, BW)).astype(bf16)


def _prep_core(xc):
    """xc: [C, H, W] fp32 -> (x_rep chunk list, x_conv bf16 [128,2,HW])

    x_rep layout: [p=(g,kj), row, gc, w] so each row range is contiguous."""
    xpad = np.zeros((C, HP, WP), np.float32)
    xpad[:, PAD:PAD + H, PAD:PAD + W] = xc
    xg = xpad.reshape(G, GC, HP, WP)
    arr = np.empty((G, NKJ, GC, HP, W), np.float32)
    for kj in range(NKJ):
        arr[:, kj] = xg[:, :, :, kj:kj + W]
    x_rep = arr.reshape(NP, GC, HP, W).astype(bf16)
    xb0 = np.ascontiguousarray(x_rep[:, :, :XB0R])
    chunks = [
        np.ascontiguousarray(x_rep[:, :, r0:r1]).reshape(NP, (r1 - r0) * GC * W)
        for (r0, r1) in CHUNKS
    ]
    x_conv = np.ascontiguousarray(
        xc.reshape(2, 128, HW).transpose(1, 0, 2)
    ).astype(bf16)
    return chunks, xb0, x_conv


def kernel(x, w1, b1, g1, be1, m1, v1, w2, b2, g2, be2, m2, v2, _profile=False):
    from concourse.bass_utils import run_bass_kernel_spmd

    if "nc" not in _CACHE:
        _CACHE["nc"] = _build_nc()
    nc = _CACHE["nc"]

    x = np.asarray(x, np.float32)
    w1t, b1p, w2t, b2t, selm, dense = _prep_weights(
        np.asarray(w1, np.float32), np.asarray(b1, np.float32),
        np.asarray(g1, np.float32), np.asarray(be1, np.float32),
        np.asarray(m1, np.float32), np.asarray(v1, np.float32),
        np.asarray(w2, np.float32), np.asarray(b2, np.float32),
        np.asarray(g2, np.float32), np.asarray(be2, np.float32),
        np.asarray(m2, np.float32), np.asarray(v2, np.float32),
    )

    in_maps = []
    for c in range(B):
        xc = x[c].reshape(C, H, W)
        chunks, xb0c, x_conv = _prep_core(xc)
        wghc = _host_wgh(xc, dense)
        im = {
            "x_conv": x_conv,
            "x_b0": xb0c,
            "w1t": w1t, "w2t": w2t, "sel": selm, "b1": b1p, "b2": b2t,
        }
        for wb in range(NWGH):
            im[f"wgh{wb}"] = np.ascontiguousarray(wghc[:, wb])
        for ci, ch in enumerate(chunks):
            im[f"x_rep{ci}"] = ch
        in_maps.append(im)

    res = run_bass_kernel_spmd(
        nc, in_maps, core_ids=list(range(8)), trace=_profile
    )
    # channel for out position (j, g, i4): invert the _strip mapping
    # (i4 = 2s + ir; quarter = 2s + j%2; qq = j//2; chan = 16g + 4q + 2qq + ir)
    chan = np.empty((4, G, 4), np.int64)
    for j in range(4):
        for g in range(G):
            for i4 in range(4):
                s, ir = i4 // 2, i4 % 2
                chan[j, g, i4] = 16 * g + 4 * (2 * s + j % 2) + 2 * (j // 2) + ir
    inv = np.argsort(chan.reshape(-1))

    outs = []
    for c in range(B):
        arr = res.results[c]["out"]  # [4, 16, 4, HW] bf16: [j, g, i4, hw]
        arr = arr.astype(np.float32).reshape(C, HW)[inv]
        outs.append(arr)
    outp = np.stack(outs, axis=0)
    if _profile:
        _CACHE["last_result"] = res
    return outp.reshape(B, C, H, W).astype(np.float32)
